# revision 13
# baseline (speedup 1.0000x reference)
"""GAT attention head (gnn_message_passing) on 8 TRN2 NeuronCores.

Strategy (dst-sharded, one AllGather):
  - Node features sharded across cores (6250 nodes each). Each core computes
    h' = x @ W for its shard plus per-node attention scalars e_dst/e_src
    (via W@a folded into an extended weight matrix), packs rows
    [h'+output_bias (128) | e_dst+b_dst | e_src+b_src | 1.0 | 0] as bf16,
    and AllGathers the full 50000-row table T.
  - Edges are sharded by destination range and sorted into 128-dst windows
    (host-side index prep). Per 128-edge chunk the core gathers the table
    rows by src ([128,1] indirect DMA - one row per partition), forms the
    score matrix F[e,j] = exp(leakyrelu(e_src_e + e_dst_j)) from the
    gathered e_src column and a per-window broadcast of the local e_dst
    values (rank-1 matmul), masks it with the one-hot dst-selection matrix
    (single fused DVE op builds (iota==dstrel)*mask), and accumulates
    Sel^T @ [msg|..|1] into a PSUM window accumulator - giving the weighted
    message sum (cols 0:128) and the softmax denominator (col 130).
  - Window epilogue: out = elu(num / max(den,1e-12)); output_bias is folded
    into the table rows (sum(alpha*(h'+bias))/den == num/den + bias).
  - No softmax max-subtraction: scores are O(1) so exp() is safe and
    softmax is shift-invariant.
Output: each core writes its 6250-row slab; host concatenates.
"""

import os
import sys

for _p in ("/opt/trn_rl_repo", "/root/.axon_site/_ro/trn_rl_repo"):
    if os.path.isdir(_p) and _p not in sys.path:
        sys.path.append(_p)

import numpy as np
import ml_dtypes

import concourse.bass as bass
import concourse.mybir as mybir
import concourse.tile as tile
from concourse import bacc
from concourse.bass import IndirectOffsetOnAxis
from concourse.bass_utils import run_bass_kernel_spmd

NC_ = 8
N = 50000
E = 800000
IN_DIM = 256
OUT_DIM = 128
NSH = N // NC_           # 6250 nodes per core
WIN = 128                # dst window size
NWIN = (NSH + WIN - 1) // WIN   # 49
TW = 132                 # table row width
F32 = mybir.dt.float32
BF16 = mybir.dt.bfloat16
I32 = mybir.dt.int32

LAST_EXEC_NS = None

_GRAPH_CACHE = {}


def _prep_edges(edge_src, edge_dst):
    """Partition edges by dst range, sort into windows, pad to a chunk
    structure (CW chunks per window) shared by all cores."""
    edge_src = np.asarray(edge_src).astype(np.int64)
    edge_dst = np.asarray(edge_dst).astype(np.int64)
    core = edge_dst // NSH
    per_core = []
    CW = np.zeros(NWIN, dtype=np.int64)
    for k in range(NC_):
        m = core == k
        s = edge_src[m]
        d = edge_dst[m] - k * NSH
        w = d // WIN
        order = np.argsort(w, kind="stable")
        per_core.append((s[order], d[order], w[order]))
        cnt = np.bincount(w, minlength=NWIN)
        CW = np.maximum(CW, (cnt + 127) // 128)
    CW = np.maximum(CW, 1)
    C = int(CW.sum())
    offs = np.zeros(NWIN + 1, dtype=np.int64)
    offs[1:] = np.cumsum(CW) * 128

    maps = []
    for k in range(NC_):
        s, d, w = per_core[k]
        srcidx = np.zeros(C * 128, np.int32)
        dstrel = np.zeros(C * 128, np.float32)
        maskv = np.zeros(C * 128, np.float32)
        cnt = np.bincount(w, minlength=NWIN)
        cum = np.zeros(NWIN + 1, dtype=np.int64)
        cum[1:] = np.cumsum(cnt)
        pos = offs[w] + (np.arange(len(s)) - cum[w])
        srcidx[pos] = s
        dstrel[pos] = (d - w * WIN).astype(np.float32)
        maskv[pos] = 1.0
        maps.append({
            "srcidx": np.ascontiguousarray(srcidx.reshape(C, 128).T),
            "dstrel": np.ascontiguousarray(dstrel.reshape(C, 128).T),
            "maskt": np.ascontiguousarray(
                (-30000.0 * (1.0 - maskv)).astype(np.float32).reshape(C, 128).T),
        })
    win_of = np.repeat(np.arange(NWIN), CW)
    return tuple(CW.tolist()), C, win_of, maps


def _build(CW, C, win_of):
    nc = bacc.Bacc("TRN2", target_bir_lowering=False, debug=False,
                   enable_asserts=True, num_devices=NC_)
    xT = nc.dram_tensor("xT", [IN_DIM, NSH], BF16, kind="ExternalInput").ap()
    wext = nc.dram_tensor("wext", [IN_DIM, TW], BF16, kind="ExternalInput").ap()
    biast = nc.dram_tensor("biast", [128, TW], F32, kind="ExternalInput").ap()
    iota = nc.dram_tensor("iota", [128, 128], BF16, kind="ExternalInput").ap()
    ones_r = nc.dram_tensor("ones_r", [1, 128], F32, kind="ExternalInput").ap()
    srcidx = nc.dram_tensor("srcidx", [128, C], I32, kind="ExternalInput").ap()
    dstrel = nc.dram_tensor("dstrel", [128, C], F32, kind="ExternalInput").ap()
    maskt = nc.dram_tensor("maskt", [128, C], F32, kind="ExternalInput").ap()
    out = nc.dram_tensor("out", [NSH, OUT_DIM], F32, kind="ExternalOutput").ap()

    ag_in = nc.dram_tensor("ag_in", [NSH, TW], BF16)
    edloc = nc.dram_tensor("edloc", [NWIN * WIN, 1], F32)   # padded e_dst column
    T = nc.dram_tensor("t_full", [N, TW], BF16, addr_space="Shared")

    EXP = mybir.ActivationFunctionType.Exp
    AO = mybir.AluOpType
    NT = NWIN  # node tiles of 128 in this core's shard (48*128 + 106)

    first_of = {}
    last_of = {}
    for c, w in enumerate(win_of):
        if w not in first_of:
            first_of[w] = c
        last_of[w] = c

    with tile.TileContext(nc) as tc:
        with tc.tile_pool(name="const", bufs=1) as constp, \
             tc.tile_pool(name="idx", bufs=1) as idxp:
            wext_t = constp.tile([128, 2 * TW], BF16)
            nc.sync.dma_start(wext_t[:, 0:TW], wext[0:128, :])
            nc.sync.dma_start(wext_t[:, TW:2 * TW], wext[128:256, :])
            biast_t = constp.tile([128, TW], F32)
            nc.sync.dma_start(biast_t[:], biast[:, :])
            iota_t = constp.tile([128, 128], BF16)
            nc.sync.dma_start(iota_t[:], iota[:, :])
            ones_t = constp.tile([1, 128], F32)
            nc.sync.dma_start(ones_t[:], ones_r[:, :])
            srcidx_t = idxp.tile([128, C], I32)
            nc.sync.dma_start(srcidx_t[:], srcidx[:, :])
            dstrel_t = idxp.tile([128, C], F32)
            nc.sync.dma_start(dstrel_t[:], dstrel[:, :])
            mask_t = idxp.tile([128, C], F32)
            nc.sync.dma_start(mask_t[:], maskt[:, :])

            # ---- phase 1: h' + table build + AllGather ----
            with tc.tile_pool(name="p1x", bufs=1) as p1x, \
                 tc.tile_pool(name="p1t", bufs=3) as p1t, \
                 tc.tile_pool(name="ps1", bufs=4, space="PSUM") as ps1:
                xt = p1x.tile([128, 2 * NSH], BF16)
                nc.sync.dma_start(xt[:, 0:NSH], xT[0:128, :])
                nc.sync.dma_start(xt[:, NSH:2 * NSH], xT[128:256, :])
                edcols = p1x.tile([128, NWIN], F32)
                nc.vector.memset(edcols[:], 0.0)
                tbig = p1x.tile([128, NWIN * TW], BF16)
                for m in range(NT):
                    pm = min(128, NSH - m * 128)
                    ps = ps1.tile([128, TW], F32, tag="ps")
                    nc.tensor.matmul(out=ps[:pm, :],
                                     lhsT=xt[:, m * 128: m * 128 + pm],
                                     rhs=wext_t[:, 0:TW], start=True, stop=False)
                    nc.tensor.matmul(out=ps[:pm, :],
                                     lhsT=xt[:, NSH + m * 128: NSH + m * 128 + pm],
                                     rhs=wext_t[:, TW:2 * TW], start=False, stop=True)
                    nc.vector.tensor_tensor(tbig[:pm, m * TW:(m + 1) * TW],
                                            ps[:pm, :], biast_t[:pm, :], op=AO.add)
                    nc.vector.tensor_tensor(edcols[:pm, m:m + 1], ps[:pm, 128:129],
                                            biast_t[:pm, 128:129], op=AO.add)
                # table tile m holds rows m*128+p at [p, m*TW:(m+1)*TW];
                # ag_in is row-major [6250, 132]: write via 3D APs in blocks
                # of 12 tiles so the AllGather input lands incrementally and
                # the collective can trigger right after the last block. The
                # last tile only has 106 valid rows, so write it separately.
                for g0 in range(0, NWIN - 1, 12):
                    g1 = min(g0 + 12, NWIN - 1)
                    nc.sync.dma_start(
                        ag_in.ap()[g0 * 128:g1 * 128, :].rearrange(
                            "(m p) e -> p m e", p=128),
                        tbig[:].rearrange("p (m e) -> p m e", e=TW)[:, g0:g1, :])
                nc.sync.dma_start(ag_in[(NWIN - 1) * 128:NSH, :],
                                  tbig[:106, (NWIN - 1) * TW:NWIN * TW])
                # node m*128+p lives at edcols[p, m]; edloc is node-flat
                nc.sync.dma_start(
                    edloc.ap().rearrange("(m p) one -> p (m one)", p=128),
                    edcols[:])

            nc.gpsimd.collective_compute(
                "AllGather", AO.bypass,
                replica_groups=[list(range(NC_))],
                ins=[ag_in.ap().opt()],
                outs=[T.ap().opt()],
            )

            # ---- phases 2+3: gather, score, accumulate, evacuate ----
            with tc.tile_pool(name="gath", bufs=16) as gp, \
                 tc.tile_pool(name="wrow", bufs=3) as wrp, \
                 tc.tile_pool(name="wbc", bufs=3) as wbp, \
                 tc.tile_pool(name="sc", bufs=8) as scp, \
                 tc.tile_pool(name="psB", bufs=2, space="PSUM") as psB, \
                 tc.tile_pool(name="ps2", bufs=3, space="PSUM") as ps2, \
                 tc.tile_pool(name="evac", bufs=2) as ev:
                psw = None
                edw_b = None
                for c in range(C):
                    w = int(win_of[c])
                    if first_of[w] == c:
                        # per-window: broadcast e_dst row to all partitions
                        edr = wrp.tile([1, WIN], F32, tag="edr")
                        edloc_rows = edloc.ap().rearrange(
                            "(a b) one -> a (b one)", b=WIN)
                        nc.sync.dma_start(edr[:], edloc_rows[w:w + 1, :])
                        edp = psB.tile([128, WIN], F32, tag="edp")
                        nc.tensor.matmul(out=edp[:], lhsT=ones_t[:], rhs=edr[:],
                                         start=True, stop=True)
                        edw_b = wbp.tile([128, WIN], F32, tag="edw")
                        nc.vector.tensor_copy(edw_b[:], edp[:])
                        psw = ps2.tile([128, TW], F32, tag="psw")
                    # per-chunk: gather 128 table rows by src
                    msg = gp.tile([128, TW], BF16, tag="msg")
                    nc.gpsimd.indirect_dma_start(
                        out=msg[:], out_offset=None, in_=T.ap(),
                        in_offset=IndirectOffsetOnAxis(
                            ap=srcidx_t[:, c: c + 1], axis=0))
                    # scores: F = exp(leaky(e_src_e + e_dst_j + maskbias_e))
                    esf = scp.tile([128, 1], F32, tag="esf")
                    nc.vector.tensor_copy(esf[:], msg[:, 129:130])
                    s0 = scp.tile([128, WIN], F32, tag="s0")
                    nc.vector.tensor_scalar(s0[:], edw_b[:], esf[:, 0:1],
                                            mask_t[:, c: c + 1],
                                            op0=AO.add, op1=AO.add)
                    s1 = scp.tile([128, WIN], F32, tag="s1")
                    nc.vector.scalar_tensor_tensor(s1[:], s0[:], 0.2, s0[:],
                                                   op0=AO.mult, op1=AO.max)
                    fm = scp.tile([128, WIN], BF16, tag="fm")
                    nc.scalar.activation(fm[:], s1[:], EXP)
                    selw = scp.tile([128, WIN], BF16, tag="selw")
                    nc.vector.scalar_tensor_tensor(selw[:], iota_t[:],
                                                   dstrel_t[:, c: c + 1],
                                                   fm[:], op0=AO.is_equal,
                                                   op1=AO.mult)
                    nc.tensor.matmul(out=psw[:], lhsT=selw[:], rhs=msg[:],
                                     start=(first_of[w] == c),
                                     stop=(last_of[w] == c))
                    if last_of[w] == c:
                        pw = min(128, NSH - w * 128)
                        den = ev.tile([128, 1], F32, tag="den")
                        nc.vector.tensor_scalar(den[:], psw[:, 130:131],
                                                1e-12, None, op0=AO.max)
                        rec = ev.tile([128, 1], F32, tag="rec")
                        nc.vector.reciprocal(rec[:], den[:])
                        o1 = ev.tile([128, 128], F32, tag="o1")
                        nc.vector.tensor_scalar(o1[:], psw[:, 0:128],
                                                rec[:, 0:1], None, op0=AO.mult)
                        mng = ev.tile([128, 128], F32, tag="mng")
                        nc.vector.tensor_scalar(mng[:], o1[:], 0.0, None,
                                                op0=AO.min)
                        eng = ev.tile([128, 128], F32, tag="eng")
                        nc.scalar.activation(eng[:], mng[:], EXP)
                        fin = ev.tile([128, 128], F32, tag="fin")
                        nc.vector.scalar_tensor_tensor(fin[:], o1[:], 0.0,
                                                       eng[:], op0=AO.max,
                                                       op1=AO.add)
                        fin2 = ev.tile([128, 128], F32, tag="fin2")
                        nc.vector.tensor_scalar(fin2[:], fin[:], 1.0, None,
                                                op0=AO.subtract)
                        nc.sync.dma_start(out[w * 128: w * 128 + pw, :],
                                          fin2[:pw, :])
    nc.compile()
    return nc


def _host_inputs(inputs):
    x = np.ascontiguousarray(np.asarray(inputs["inputs"], dtype=np.float32))
    edge_src = np.asarray(inputs["edge_src"])
    edge_dst = np.asarray(inputs["edge_dst"])
    W = np.asarray(inputs["W_seq"], dtype=np.float32)
    a_dst = np.asarray(inputs["a_dst"], dtype=np.float32)
    b_dst = np.float32(inputs["b_dst"])
    a_src = np.asarray(inputs["a_src"], dtype=np.float32)
    b_src = np.float32(inputs["b_src"])
    output_bias = np.asarray(inputs["output_bias"], dtype=np.float32)

    CW, C, win_of, edge_maps = _prep_edges(edge_src, edge_dst)

    wext = np.zeros((IN_DIM, TW), np.float32)
    wext[:, 0:OUT_DIM] = W
    wext[:, 128] = W @ a_dst
    wext[:, 129] = W @ a_src
    wext = wext.astype(ml_dtypes.bfloat16)
    bias_ext = np.zeros(TW, np.float32)
    bias_ext[0:OUT_DIM] = output_bias
    bias_ext[128] = b_dst
    bias_ext[129] = b_src
    bias_ext[130] = 1.0
    biast = np.ascontiguousarray(np.tile(bias_ext[None, :], (128, 1)))
    iota = np.ascontiguousarray(
        np.tile(np.arange(128, dtype=np.float32)[None, :], (128, 1))
    ).astype(ml_dtypes.bfloat16)
    ones_r = np.ones((1, 128), np.float32)

    in_maps = []
    for k in range(NC_):
        m = {
            "xT": np.ascontiguousarray(
                x[k * NSH:(k + 1) * NSH].T).astype(ml_dtypes.bfloat16),
            "wext": wext,
            "biast": biast,
            "iota": iota,
            "ones_r": ones_r,
        }
        m.update(edge_maps[k])
        in_maps.append(m)
    return CW, C, win_of, in_maps


def kernel(**inputs) -> np.ndarray:
    global LAST_EXEC_NS
    CW, C, win_of, in_maps = _host_inputs(inputs)
    key = (CW, C)
    if key not in _GRAPH_CACHE:
        _GRAPH_CACHE[key] = _build(CW, C, win_of)
    nc = _GRAPH_CACHE[key]

    want_trace = bool(int(os.environ.get("KERNEL_TRACE", "0")))
    try:
        res = run_bass_kernel_spmd(nc, in_maps, core_ids=list(range(NC_)),
                                   trace=want_trace)
    except Exception:
        if not want_trace:
            raise
        res = run_bass_kernel_spmd(nc, in_maps, core_ids=list(range(NC_)),
                                   trace=False)
    LAST_EXEC_NS = res.exec_time_ns
    out = np.concatenate([res.results[k]["out"] for k in range(NC_)], axis=0)
    return out.astype(np.float32)


# revision 14
# speedup vs baseline: 1.1659x; 1.1659x over previous
"""GAT attention head (gnn_message_passing) on 8 TRN2 NeuronCores.

Strategy (dst-sharded, one AllGather):
  - Node features sharded across cores (6250 nodes each). Each core computes
    h' = x @ W for its shard plus per-node attention scalars e_dst/e_src
    (via W@a folded into an extended weight matrix), packs rows
    [h'+output_bias (128) | e_dst+b_dst | e_src+b_src | 1.0 | 0] as bf16,
    and AllGathers the full 50000-row table T.
  - Edges are sharded by destination range and sorted into 128-dst windows
    (host-side index prep). Per 128-edge chunk the core gathers the table
    rows by src ([128,1] indirect DMA - one row per partition), forms the
    score matrix F[e,j] = exp(leakyrelu(e_src_e + e_dst_j)) from the
    gathered e_src column and a per-window broadcast of the local e_dst
    values (rank-1 matmul), masks it with the one-hot dst-selection matrix
    (single fused DVE op builds (iota==dstrel)*mask), and accumulates
    Sel^T @ [msg|..|1] into a PSUM window accumulator - giving the weighted
    message sum (cols 0:128) and the softmax denominator (col 130).
  - Window epilogue: out = elu(num / max(den,1e-12)); output_bias is folded
    into the table rows (sum(alpha*(h'+bias))/den == num/den + bias).
  - No softmax max-subtraction: scores are O(1) so exp() is safe and
    softmax is shift-invariant.
Output: each core writes its 6250-row slab; host concatenates.
"""

import os
import sys

for _p in ("/opt/trn_rl_repo", "/root/.axon_site/_ro/trn_rl_repo"):
    if os.path.isdir(_p) and _p not in sys.path:
        sys.path.append(_p)

import numpy as np
import ml_dtypes

import concourse.bass as bass
import concourse.mybir as mybir
import concourse.tile as tile
from concourse import bacc
from concourse.bass import IndirectOffsetOnAxis
from concourse.bass_utils import run_bass_kernel_spmd

NC_ = 8
N = 50000
E = 800000
IN_DIM = 256
OUT_DIM = 128
NSH = N // NC_           # 6250 nodes per core
WIN = 128                # dst window size
NWIN = (NSH + WIN - 1) // WIN   # 49
TW = 132                 # table row width
F32 = mybir.dt.float32
BF16 = mybir.dt.bfloat16
I32 = mybir.dt.int32

LAST_EXEC_NS = None

_GRAPH_CACHE = {}


def _prep_edges(edge_src, edge_dst):
    """Partition edges by dst range, sort into windows, pad to a chunk
    structure (CW chunks per window) shared by all cores."""
    edge_src = np.asarray(edge_src).astype(np.int64)
    edge_dst = np.asarray(edge_dst).astype(np.int64)
    core = edge_dst // NSH
    per_core = []
    CW = np.zeros(NWIN, dtype=np.int64)
    for k in range(NC_):
        m = core == k
        s = edge_src[m]
        d = edge_dst[m] - k * NSH
        w = d // WIN
        order = np.argsort(w, kind="stable")
        per_core.append((s[order], d[order], w[order]))
        cnt = np.bincount(w, minlength=NWIN)
        CW = np.maximum(CW, (cnt + 127) // 128)
    CW = np.maximum(CW, 1)
    C = int(CW.sum())
    offs = np.zeros(NWIN + 1, dtype=np.int64)
    offs[1:] = np.cumsum(CW) * 128

    maps = []
    for k in range(NC_):
        s, d, w = per_core[k]
        srcidx = np.zeros(C * 128, np.int32)
        dstrel = np.zeros(C * 128, np.float32)
        maskv = np.zeros(C * 128, np.float32)
        cnt = np.bincount(w, minlength=NWIN)
        cum = np.zeros(NWIN + 1, dtype=np.int64)
        cum[1:] = np.cumsum(cnt)
        pos = offs[w] + (np.arange(len(s)) - cum[w])
        srcidx[pos] = s
        dstrel[pos] = (d - w * WIN).astype(np.float32)
        maskv[pos] = 1.0
        maps.append({
            "srcidx": np.ascontiguousarray(srcidx.reshape(C, 128).T),
            "dstrel": np.ascontiguousarray(dstrel.reshape(C, 128).T),
            "maskt": np.ascontiguousarray(
                (-30000.0 * (1.0 - maskv)).astype(np.float32).reshape(C, 128).T),
        })
    win_of = np.repeat(np.arange(NWIN), CW)
    return tuple(CW.tolist()), C, win_of, maps


def _build(CW, C, win_of):
    nc = bacc.Bacc("TRN2", target_bir_lowering=False, debug=False,
                   enable_asserts=True, num_devices=NC_)
    xT = nc.dram_tensor("xT", [IN_DIM, NSH], BF16, kind="ExternalInput").ap()
    wext = nc.dram_tensor("wext", [IN_DIM, TW], BF16, kind="ExternalInput").ap()
    biast = nc.dram_tensor("biast", [128, TW], F32, kind="ExternalInput").ap()
    iota = nc.dram_tensor("iota", [128, 128], BF16, kind="ExternalInput").ap()
    ones_r = nc.dram_tensor("ones_r", [1, 128], F32, kind="ExternalInput").ap()
    srcidx = nc.dram_tensor("srcidx", [128, C], I32, kind="ExternalInput").ap()
    dstrel = nc.dram_tensor("dstrel", [128, C], F32, kind="ExternalInput").ap()
    maskt = nc.dram_tensor("maskt", [128, C], F32, kind="ExternalInput").ap()
    out = nc.dram_tensor("out", [NSH, OUT_DIM], F32, kind="ExternalOutput").ap()

    ag_in = nc.dram_tensor("ag_in", [NSH, TW], BF16)
    edloc = nc.dram_tensor("edloc", [NWIN * WIN, 1], F32)   # padded e_dst column
    T = nc.dram_tensor("t_full", [N, TW], BF16, addr_space="Shared")

    EXP = mybir.ActivationFunctionType.Exp
    AO = mybir.AluOpType
    NT = NWIN  # node tiles of 128 in this core's shard (48*128 + 106)

    first_of = {}
    last_of = {}
    for c, w in enumerate(win_of):
        if w not in first_of:
            first_of[w] = c
        last_of[w] = c

    with tile.TileContext(nc) as tc:
        with tc.tile_pool(name="const", bufs=1) as constp, \
             tc.tile_pool(name="idx", bufs=1) as idxp:
            wext_t = constp.tile([128, 2 * TW], BF16)
            nc.sync.dma_start(wext_t[:, 0:TW], wext[0:128, :])
            nc.sync.dma_start(wext_t[:, TW:2 * TW], wext[128:256, :])
            biast_t = constp.tile([128, TW], F32)
            nc.sync.dma_start(biast_t[:], biast[:, :])
            iota_t = constp.tile([128, 128], BF16)
            nc.sync.dma_start(iota_t[:], iota[:, :])
            ones_t = constp.tile([1, 128], F32)
            nc.sync.dma_start(ones_t[:], ones_r[:, :])
            srcidx_t = idxp.tile([128, C], I32)
            nc.sync.dma_start(srcidx_t[:], srcidx[:, :])
            dstrel_t = idxp.tile([128, C], F32)
            nc.sync.dma_start(dstrel_t[:], dstrel[:, :])
            mask_t = idxp.tile([128, C], F32)
            nc.sync.dma_start(mask_t[:], maskt[:, :])

            # ---- phase 1: h' + table build + AllGather ----
            with tc.tile_pool(name="p1x", bufs=1) as p1x, \
                 tc.tile_pool(name="p1t", bufs=3) as p1t, \
                 tc.tile_pool(name="ps1", bufs=4, space="PSUM") as ps1:
                xt = p1x.tile([128, 2 * NSH], BF16)
                nc.sync.dma_start(xt[:, 0:NSH], xT[0:128, :])
                nc.sync.dma_start(xt[:, NSH:2 * NSH], xT[128:256, :])
                edcols = p1x.tile([128, NWIN], F32)
                nc.vector.memset(edcols[:], 0.0)
                tbig = p1x.tile([128, NWIN * TW], BF16)
                for m in range(NT):
                    pm = min(128, NSH - m * 128)
                    ps = ps1.tile([128, TW], F32, tag="ps")
                    nc.tensor.matmul(out=ps[:pm, :],
                                     lhsT=xt[:, m * 128: m * 128 + pm],
                                     rhs=wext_t[:, 0:TW], start=True, stop=False)
                    nc.tensor.matmul(out=ps[:pm, :],
                                     lhsT=xt[:, NSH + m * 128: NSH + m * 128 + pm],
                                     rhs=wext_t[:, TW:2 * TW], start=False, stop=True)
                    nc.vector.tensor_tensor(tbig[:pm, m * TW:(m + 1) * TW],
                                            ps[:pm, :], biast_t[:pm, :], op=AO.add)
                    nc.vector.tensor_tensor(edcols[:pm, m:m + 1], ps[:pm, 128:129],
                                            biast_t[:pm, 128:129], op=AO.add)
                # table tile m holds rows m*128+p at [p, m*TW:(m+1)*TW];
                # ag_in is row-major [6250, 132]: write via 3D AP. The last
                # tile only has 106 valid rows, so write it separately.
                nc.sync.dma_start(
                    ag_in.ap()[0:(NWIN - 1) * 128, :].rearrange(
                        "(m p) e -> p m e", p=128),
                    tbig[:].rearrange("p (m e) -> p m e", e=TW)[:, 0:NWIN - 1, :])
                nc.sync.dma_start(ag_in[(NWIN - 1) * 128:NSH, :],
                                  tbig[:106, (NWIN - 1) * TW:NWIN * TW])
                # node m*128+p lives at edcols[p, m]; edloc is node-flat
                nc.sync.dma_start(
                    edloc.ap().rearrange("(m p) one -> p (m one)", p=128),
                    edcols[:])

            nc.gpsimd.collective_compute(
                "AllGather", AO.bypass,
                replica_groups=[list(range(NC_))],
                ins=[ag_in.ap().opt()],
                outs=[T.ap().opt()],
            )

            # ---- phases 2+3: gather, score, accumulate, evacuate ----
            with tc.tile_pool(name="gath", bufs=16) as gp, \
                 tc.tile_pool(name="wrow", bufs=3) as wrp, \
                 tc.tile_pool(name="wbc", bufs=3) as wbp, \
                 tc.tile_pool(name="sc", bufs=8) as scp, \
                 tc.tile_pool(name="psB", bufs=2, space="PSUM") as psB, \
                 tc.tile_pool(name="ps2", bufs=3, space="PSUM") as ps2, \
                 tc.tile_pool(name="evac", bufs=2) as ev:
                psw = None
                edw_b = None
                for c in range(C):
                    w = int(win_of[c])
                    if first_of[w] == c:
                        # per-window: broadcast e_dst row to all partitions
                        edr = wrp.tile([1, WIN], F32, tag="edr")
                        edloc_rows = edloc.ap().rearrange(
                            "(a b) one -> a (b one)", b=WIN)
                        nc.sync.dma_start(edr[:], edloc_rows[w:w + 1, :])
                        edp = psB.tile([128, WIN], F32, tag="edp")
                        nc.tensor.matmul(out=edp[:], lhsT=ones_t[:], rhs=edr[:],
                                         start=True, stop=True)
                        edw_b = wbp.tile([128, WIN], F32, tag="edw")
                        nc.vector.tensor_copy(edw_b[:], edp[:])
                        psw = ps2.tile([128, TW], F32, tag="psw")
                    # per-chunk: gather 128 table rows by src
                    msg = gp.tile([128, TW], BF16, tag="msg")
                    nc.gpsimd.indirect_dma_start(
                        out=msg[:], out_offset=None, in_=T.ap(),
                        in_offset=IndirectOffsetOnAxis(
                            ap=srcidx_t[:, c: c + 1], axis=0))
                    # scores: F = exp(leaky(e_src_e + e_dst_j + maskbias_e))
                    esf = scp.tile([128, 1], F32, tag="esf")
                    nc.vector.tensor_copy(esf[:], msg[:, 129:130])
                    s0 = scp.tile([128, WIN], F32, tag="s0")
                    nc.vector.tensor_scalar(s0[:], edw_b[:], esf[:, 0:1],
                                            mask_t[:, c: c + 1],
                                            op0=AO.add, op1=AO.add)
                    s1 = scp.tile([128, WIN], F32, tag="s1")
                    nc.vector.scalar_tensor_tensor(s1[:], s0[:], 0.2, s0[:],
                                                   op0=AO.mult, op1=AO.max)
                    fm = scp.tile([128, WIN], BF16, tag="fm")
                    nc.scalar.activation(fm[:], s1[:], EXP)
                    selw = scp.tile([128, WIN], BF16, tag="selw")
                    nc.vector.scalar_tensor_tensor(selw[:], iota_t[:],
                                                   dstrel_t[:, c: c + 1],
                                                   fm[:], op0=AO.is_equal,
                                                   op1=AO.mult)
                    nc.tensor.matmul(out=psw[:], lhsT=selw[:], rhs=msg[:],
                                     start=(first_of[w] == c),
                                     stop=(last_of[w] == c))
                    if last_of[w] == c:
                        pw = min(128, NSH - w * 128)
                        den = ev.tile([128, 1], F32, tag="den")
                        nc.vector.tensor_scalar(den[:], psw[:, 130:131],
                                                1e-12, None, op0=AO.max)
                        rec = ev.tile([128, 1], F32, tag="rec")
                        nc.vector.reciprocal(rec[:], den[:])
                        o1 = ev.tile([128, 128], F32, tag="o1")
                        nc.vector.tensor_scalar(o1[:], psw[:, 0:128],
                                                rec[:, 0:1], None, op0=AO.mult)
                        mng = ev.tile([128, 128], F32, tag="mng")
                        nc.vector.tensor_scalar(mng[:], o1[:], 0.0, None,
                                                op0=AO.min)
                        eng = ev.tile([128, 128], F32, tag="eng")
                        nc.scalar.activation(eng[:], mng[:], EXP)
                        fin = ev.tile([128, 128], F32, tag="fin")
                        nc.vector.scalar_tensor_tensor(fin[:], o1[:], 0.0,
                                                       eng[:], op0=AO.max,
                                                       op1=AO.add)
                        fin2 = ev.tile([128, 128], F32, tag="fin2")
                        nc.vector.tensor_scalar(fin2[:], fin[:], 1.0, None,
                                                op0=AO.subtract)
                        nc.sync.dma_start(out[w * 128: w * 128 + pw, :],
                                          fin2[:pw, :])
    nc.compile()
    return nc


def _host_inputs(inputs):
    x = np.ascontiguousarray(np.asarray(inputs["inputs"], dtype=np.float32))
    edge_src = np.asarray(inputs["edge_src"])
    edge_dst = np.asarray(inputs["edge_dst"])
    W = np.asarray(inputs["W_seq"], dtype=np.float32)
    a_dst = np.asarray(inputs["a_dst"], dtype=np.float32)
    b_dst = np.float32(inputs["b_dst"])
    a_src = np.asarray(inputs["a_src"], dtype=np.float32)
    b_src = np.float32(inputs["b_src"])
    output_bias = np.asarray(inputs["output_bias"], dtype=np.float32)

    CW, C, win_of, edge_maps = _prep_edges(edge_src, edge_dst)

    wext = np.zeros((IN_DIM, TW), np.float32)
    wext[:, 0:OUT_DIM] = W
    wext[:, 128] = W @ a_dst
    wext[:, 129] = W @ a_src
    wext = wext.astype(ml_dtypes.bfloat16)
    bias_ext = np.zeros(TW, np.float32)
    bias_ext[0:OUT_DIM] = output_bias
    bias_ext[128] = b_dst
    bias_ext[129] = b_src
    bias_ext[130] = 1.0
    biast = np.ascontiguousarray(np.tile(bias_ext[None, :], (128, 1)))
    iota = np.ascontiguousarray(
        np.tile(np.arange(128, dtype=np.float32)[None, :], (128, 1))
    ).astype(ml_dtypes.bfloat16)
    ones_r = np.ones((1, 128), np.float32)

    in_maps = []
    for k in range(NC_):
        m = {
            "xT": np.ascontiguousarray(
                x[k * NSH:(k + 1) * NSH].T).astype(ml_dtypes.bfloat16),
            "wext": wext,
            "biast": biast,
            "iota": iota,
            "ones_r": ones_r,
        }
        m.update(edge_maps[k])
        in_maps.append(m)
    return CW, C, win_of, in_maps


def kernel(**inputs) -> np.ndarray:
    global LAST_EXEC_NS
    CW, C, win_of, in_maps = _host_inputs(inputs)
    key = (CW, C)
    if key not in _GRAPH_CACHE:
        _GRAPH_CACHE[key] = _build(CW, C, win_of)
    nc = _GRAPH_CACHE[key]

    want_trace = bool(int(os.environ.get("KERNEL_TRACE", "0")))
    try:
        res = run_bass_kernel_spmd(nc, in_maps, core_ids=list(range(NC_)),
                                   trace=want_trace)
    except Exception:
        if not want_trace:
            raise
        res = run_bass_kernel_spmd(nc, in_maps, core_ids=list(range(NC_)),
                                   trace=False)
    LAST_EXEC_NS = res.exec_time_ns
    out = np.concatenate([res.results[k]["out"] for k in range(NC_)], axis=0)
    return out.astype(np.float32)


# revision 17
# speedup vs baseline: 1.1665x; 1.0005x over previous
"""GAT attention head (gnn_message_passing) on 8 TRN2 NeuronCores.

Strategy (dst-sharded, one AllGather):
  - Node features sharded across cores (6250 nodes each). Each core computes
    h' = x @ W for its shard plus per-node attention scalars e_dst/e_src
    (via W@a folded into an extended weight matrix), packs rows
    [h'+output_bias (128) | e_dst+b_dst | e_src+b_src | 1.0 | 0] as bf16,
    and AllGathers the full 50000-row table T.
  - Edges are sharded by destination range and sorted into 128-dst windows
    (host-side index prep). Per 128-edge chunk the core gathers the table
    rows by src ([128,1] indirect DMA - one row per partition), forms the
    score matrix F[e,j] = exp(leakyrelu(e_src_e + e_dst_j)) from the
    gathered e_src column and a per-window broadcast of the local e_dst
    values (rank-1 matmul), masks it with the one-hot dst-selection matrix
    (single fused DVE op builds (iota==dstrel)*mask), and accumulates
    Sel^T @ [msg|..|1] into a PSUM window accumulator - giving the weighted
    message sum (cols 0:128) and the softmax denominator (col 130).
  - Window epilogue: out = elu(num / max(den,1e-12)); output_bias is folded
    into the table rows (sum(alpha*(h'+bias))/den == num/den + bias).
  - No softmax max-subtraction: scores are O(1) so exp() is safe and
    softmax is shift-invariant.
Output: each core writes its 6250-row slab; host concatenates.
"""

import os
import sys

for _p in ("/opt/trn_rl_repo", "/root/.axon_site/_ro/trn_rl_repo"):
    if os.path.isdir(_p) and _p not in sys.path:
        sys.path.append(_p)

import numpy as np
import ml_dtypes

import concourse.bass as bass
import concourse.mybir as mybir
import concourse.tile as tile
from concourse import bacc
from concourse.bass import IndirectOffsetOnAxis
from concourse.bass_utils import run_bass_kernel_spmd

NC_ = 8
N = 50000
E = 800000
IN_DIM = 256
OUT_DIM = 128
NSH = N // NC_           # 6250 nodes per core
WIN = 128                # dst window size
NWIN = (NSH + WIN - 1) // WIN   # 49
TW = 132                 # table row width
F32 = mybir.dt.float32
BF16 = mybir.dt.bfloat16
I32 = mybir.dt.int32

LAST_EXEC_NS = None

_GRAPH_CACHE = {}


def _prep_edges(edge_src, edge_dst):
    """Partition edges by dst range, sort into windows, pad to a chunk
    structure (CW chunks per window) shared by all cores."""
    edge_src = np.asarray(edge_src).astype(np.int64)
    edge_dst = np.asarray(edge_dst).astype(np.int64)
    core = edge_dst // NSH
    per_core = []
    CW = np.zeros(NWIN, dtype=np.int64)
    for k in range(NC_):
        m = core == k
        s = edge_src[m]
        d = edge_dst[m] - k * NSH
        w = d // WIN
        order = np.argsort(w, kind="stable")
        per_core.append((s[order], d[order], w[order]))
        cnt = np.bincount(w, minlength=NWIN)
        CW = np.maximum(CW, (cnt + 127) // 128)
    CW = np.maximum(CW, 1)
    C = int(CW.sum())
    offs = np.zeros(NWIN + 1, dtype=np.int64)
    offs[1:] = np.cumsum(CW) * 128

    maps = []
    for k in range(NC_):
        s, d, w = per_core[k]
        srcidx = np.zeros(C * 128, np.int32)
        dstrel = np.zeros(C * 128, np.float32)
        maskv = np.zeros(C * 128, np.float32)
        cnt = np.bincount(w, minlength=NWIN)
        cum = np.zeros(NWIN + 1, dtype=np.int64)
        cum[1:] = np.cumsum(cnt)
        pos = offs[w] + (np.arange(len(s)) - cum[w])
        srcidx[pos] = s
        dstrel[pos] = (d - w * WIN).astype(np.float32)
        maskv[pos] = 1.0
        maps.append({
            "srcidx": np.ascontiguousarray(srcidx.reshape(C, 128).T),
            "dstrel": np.ascontiguousarray(dstrel.reshape(C, 128).T),
            "maskt": np.ascontiguousarray(
                (-30000.0 * (1.0 - maskv)).astype(np.float32).reshape(C, 128).T),
        })
    win_of = np.repeat(np.arange(NWIN), CW)
    return tuple(CW.tolist()), C, win_of, maps


def _build(CW, C, win_of):
    nc = bacc.Bacc("TRN2", target_bir_lowering=False, debug=False,
                   enable_asserts=True, num_devices=NC_)
    xT = nc.dram_tensor("xT", [IN_DIM, NSH], BF16, kind="ExternalInput").ap()
    wext = nc.dram_tensor("wext", [IN_DIM, TW], BF16, kind="ExternalInput").ap()
    biast = nc.dram_tensor("biast", [128, TW], F32, kind="ExternalInput").ap()
    iota = nc.dram_tensor("iota", [128, 128], BF16, kind="ExternalInput").ap()
    ones_r = nc.dram_tensor("ones_r", [1, 128], F32, kind="ExternalInput").ap()
    srcidx = nc.dram_tensor("srcidx", [128, C], I32, kind="ExternalInput").ap()
    dstrel = nc.dram_tensor("dstrel", [128, C], F32, kind="ExternalInput").ap()
    maskt = nc.dram_tensor("maskt", [128, C], F32, kind="ExternalInput").ap()
    out = nc.dram_tensor("out", [NSH, OUT_DIM], F32, kind="ExternalOutput").ap()

    ag_in = nc.dram_tensor("ag_in", [NSH, TW], BF16)
    edloc = nc.dram_tensor("edloc", [NWIN * WIN, 1], F32)   # padded e_dst column
    T = nc.dram_tensor("t_full", [N, TW], BF16, addr_space="Shared")

    EXP = mybir.ActivationFunctionType.Exp
    AO = mybir.AluOpType
    NT = NWIN  # node tiles of 128 in this core's shard (48*128 + 106)

    first_of = {}
    last_of = {}
    for c, w in enumerate(win_of):
        if w not in first_of:
            first_of[w] = c
        last_of[w] = c

    with tile.TileContext(nc) as tc:
        with tc.tile_pool(name="const", bufs=1) as constp, \
             tc.tile_pool(name="idx", bufs=1) as idxp:
            wext_t = constp.tile([128, 2 * TW], BF16)
            nc.sync.dma_start(wext_t[:, 0:TW], wext[0:128, :])
            nc.sync.dma_start(wext_t[:, TW:2 * TW], wext[128:256, :])
            biast_t = constp.tile([128, TW], F32)
            nc.sync.dma_start(biast_t[:], biast[:, :])
            iota_t = constp.tile([128, 128], BF16)
            nc.sync.dma_start(iota_t[:], iota[:, :])
            ones_t = constp.tile([1, 128], F32)
            nc.sync.dma_start(ones_t[:], ones_r[:, :])
            srcidx_t = idxp.tile([128, C], I32)
            nc.sync.dma_start(srcidx_t[:], srcidx[:, :])
            dstrel_t = idxp.tile([128, C], F32)
            nc.sync.dma_start(dstrel_t[:], dstrel[:, :])
            mask_t = idxp.tile([128, C], F32)
            nc.sync.dma_start(mask_t[:], maskt[:, :])

            # ---- phase 1: h' + table build + AllGather ----
            with tc.tile_pool(name="p1x", bufs=1) as p1x, \
                 tc.tile_pool(name="p1t", bufs=3) as p1t, \
                 tc.tile_pool(name="ps1", bufs=4, space="PSUM") as ps1:
                xt = p1x.tile([128, 2 * NSH], BF16)
                nc.sync.dma_start(xt[:, 0:NSH], xT[0:128, :])
                nc.sync.dma_start(xt[:, NSH:2 * NSH], xT[128:256, :])
                edcols = p1x.tile([128, NWIN], F32)
                nc.vector.memset(edcols[:], 0.0)
                # four independent table-block tiles so each block's ag_in
                # write can start as soon as ITS adds are done (a single big
                # tile serializes the write behind all 49 adds)
                blk_base = [0, 13, 25, 37]
                blk_len = [13, 12, 12, 12]
                tb4 = [p1x.tile([128, blk_len[b] * TW], BF16, name=f"tb4_{b}",
                                tag=f"tb4_{b}") for b in range(4)]
                for m in range(NT):
                    pm = min(128, NSH - m * 128)
                    b = 0
                    while m >= blk_base[b] + blk_len[b]:
                        b += 1
                    lm = m - blk_base[b]
                    ps = ps1.tile([128, TW], F32, tag="ps")
                    nc.tensor.matmul(out=ps[:pm, :],
                                     lhsT=xt[:, m * 128: m * 128 + pm],
                                     rhs=wext_t[:, 0:TW], start=True, stop=False)
                    nc.tensor.matmul(out=ps[:pm, :],
                                     lhsT=xt[:, NSH + m * 128: NSH + m * 128 + pm],
                                     rhs=wext_t[:, TW:2 * TW], start=False, stop=True)
                    nc.vector.tensor_tensor(tb4[b][:pm, lm * TW:(lm + 1) * TW],
                                            ps[:pm, :], biast_t[:pm, :], op=AO.add)
                    nc.vector.tensor_tensor(edcols[:pm, m:m + 1], ps[:pm, 128:129],
                                            biast_t[:pm, 128:129], op=AO.add)
                    if m == blk_base[b] + blk_len[b] - 1:
                        # block complete: write its full-128-row tiles; the
                        # 106-row tile 48 in the last block goes separately
                        nfull = blk_len[b] - (1 if b == 3 else 0)
                        nc.sync.dma_start(
                            ag_in.ap()[blk_base[b] * 128:
                                       (blk_base[b] + nfull) * 128, :].rearrange(
                                "(m p) e -> p m e", p=128),
                            tb4[b][:].rearrange(
                                "p (m e) -> p m e", e=TW)[:, 0:nfull, :])
                nc.sync.dma_start(ag_in[(NWIN - 1) * 128:NSH, :],
                                  tb4[3][:106, 11 * TW:12 * TW])
                # node m*128+p lives at edcols[p, m]; edloc is node-flat
                nc.sync.dma_start(
                    edloc.ap().rearrange("(m p) one -> p (m one)", p=128),
                    edcols[:])

            nc.gpsimd.collective_compute(
                "AllGather", AO.bypass,
                replica_groups=[list(range(NC_))],
                ins=[ag_in.ap().opt()],
                outs=[T.ap().opt()],
            )

            # ---- phases 2+3: gather, score, accumulate, evacuate ----
            with tc.tile_pool(name="gath", bufs=16) as gp, \
                 tc.tile_pool(name="wrow", bufs=3) as wrp, \
                 tc.tile_pool(name="wbc", bufs=3) as wbp, \
                 tc.tile_pool(name="sc", bufs=8) as scp, \
                 tc.tile_pool(name="psB", bufs=2, space="PSUM") as psB, \
                 tc.tile_pool(name="ps2", bufs=3, space="PSUM") as ps2, \
                 tc.tile_pool(name="evac", bufs=2) as ev:
                psw = None
                edw_b = None
                for c in range(C):
                    w = int(win_of[c])
                    if first_of[w] == c:
                        # per-window: broadcast e_dst row to all partitions
                        edr = wrp.tile([1, WIN], F32, tag="edr")
                        edloc_rows = edloc.ap().rearrange(
                            "(a b) one -> a (b one)", b=WIN)
                        nc.sync.dma_start(edr[:], edloc_rows[w:w + 1, :])
                        edp = psB.tile([128, WIN], F32, tag="edp")
                        nc.tensor.matmul(out=edp[:], lhsT=ones_t[:], rhs=edr[:],
                                         start=True, stop=True)
                        edw_b = wbp.tile([128, WIN], F32, tag="edw")
                        nc.vector.tensor_copy(edw_b[:], edp[:])
                        psw = ps2.tile([128, TW], F32, tag="psw")
                    # per-chunk: gather 128 table rows by src
                    msg = gp.tile([128, TW], BF16, tag="msg")
                    nc.gpsimd.indirect_dma_start(
                        out=msg[:], out_offset=None, in_=T.ap(),
                        in_offset=IndirectOffsetOnAxis(
                            ap=srcidx_t[:, c: c + 1], axis=0))
                    # scores: F = exp(leaky(e_src_e + e_dst_j + maskbias_e))
                    esf = scp.tile([128, 1], F32, tag="esf")
                    nc.vector.tensor_copy(esf[:], msg[:, 129:130])
                    s0 = scp.tile([128, WIN], F32, tag="s0")
                    nc.vector.tensor_scalar(s0[:], edw_b[:], esf[:, 0:1],
                                            mask_t[:, c: c + 1],
                                            op0=AO.add, op1=AO.add)
                    s1 = scp.tile([128, WIN], F32, tag="s1")
                    nc.vector.scalar_tensor_tensor(s1[:], s0[:], 0.2, s0[:],
                                                   op0=AO.mult, op1=AO.max)
                    fm = scp.tile([128, WIN], BF16, tag="fm")
                    nc.scalar.activation(fm[:], s1[:], EXP)
                    selw = scp.tile([128, WIN], BF16, tag="selw")
                    nc.vector.scalar_tensor_tensor(selw[:], iota_t[:],
                                                   dstrel_t[:, c: c + 1],
                                                   fm[:], op0=AO.is_equal,
                                                   op1=AO.mult)
                    nc.tensor.matmul(out=psw[:], lhsT=selw[:], rhs=msg[:],
                                     start=(first_of[w] == c),
                                     stop=(last_of[w] == c))
                    if last_of[w] == c:
                        pw = min(128, NSH - w * 128)
                        den = ev.tile([128, 1], F32, tag="den")
                        nc.vector.tensor_scalar(den[:], psw[:, 130:131],
                                                1e-12, None, op0=AO.max)
                        rec = ev.tile([128, 1], F32, tag="rec")
                        nc.vector.reciprocal(rec[:], den[:])
                        o1 = ev.tile([128, 128], F32, tag="o1")
                        nc.vector.tensor_scalar(o1[:], psw[:, 0:128],
                                                rec[:, 0:1], None, op0=AO.mult)
                        mng = ev.tile([128, 128], F32, tag="mng")
                        nc.vector.tensor_scalar(mng[:], o1[:], 0.0, None,
                                                op0=AO.min)
                        eng = ev.tile([128, 128], F32, tag="eng")
                        nc.scalar.activation(eng[:], mng[:], EXP)
                        fin = ev.tile([128, 128], F32, tag="fin")
                        nc.vector.scalar_tensor_tensor(fin[:], o1[:], 0.0,
                                                       eng[:], op0=AO.max,
                                                       op1=AO.add)
                        fin2 = ev.tile([128, 128], F32, tag="fin2")
                        nc.vector.tensor_scalar(fin2[:], fin[:], 1.0, None,
                                                op0=AO.subtract)
                        nc.sync.dma_start(out[w * 128: w * 128 + pw, :],
                                          fin2[:pw, :])
    nc.compile()
    return nc


def _host_inputs(inputs):
    x = np.ascontiguousarray(np.asarray(inputs["inputs"], dtype=np.float32))
    edge_src = np.asarray(inputs["edge_src"])
    edge_dst = np.asarray(inputs["edge_dst"])
    W = np.asarray(inputs["W_seq"], dtype=np.float32)
    a_dst = np.asarray(inputs["a_dst"], dtype=np.float32)
    b_dst = np.float32(inputs["b_dst"])
    a_src = np.asarray(inputs["a_src"], dtype=np.float32)
    b_src = np.float32(inputs["b_src"])
    output_bias = np.asarray(inputs["output_bias"], dtype=np.float32)

    CW, C, win_of, edge_maps = _prep_edges(edge_src, edge_dst)

    wext = np.zeros((IN_DIM, TW), np.float32)
    wext[:, 0:OUT_DIM] = W
    wext[:, 128] = W @ a_dst
    wext[:, 129] = W @ a_src
    wext = wext.astype(ml_dtypes.bfloat16)
    bias_ext = np.zeros(TW, np.float32)
    bias_ext[0:OUT_DIM] = output_bias
    bias_ext[128] = b_dst
    bias_ext[129] = b_src
    bias_ext[130] = 1.0
    biast = np.ascontiguousarray(np.tile(bias_ext[None, :], (128, 1)))
    iota = np.ascontiguousarray(
        np.tile(np.arange(128, dtype=np.float32)[None, :], (128, 1))
    ).astype(ml_dtypes.bfloat16)
    ones_r = np.ones((1, 128), np.float32)

    in_maps = []
    for k in range(NC_):
        m = {
            "xT": np.ascontiguousarray(
                x[k * NSH:(k + 1) * NSH].T).astype(ml_dtypes.bfloat16),
            "wext": wext,
            "biast": biast,
            "iota": iota,
            "ones_r": ones_r,
        }
        m.update(edge_maps[k])
        in_maps.append(m)
    return CW, C, win_of, in_maps


def kernel(**inputs) -> np.ndarray:
    global LAST_EXEC_NS
    CW, C, win_of, in_maps = _host_inputs(inputs)
    key = (CW, C)
    if key not in _GRAPH_CACHE:
        _GRAPH_CACHE[key] = _build(CW, C, win_of)
    nc = _GRAPH_CACHE[key]

    want_trace = bool(int(os.environ.get("KERNEL_TRACE", "0")))
    try:
        res = run_bass_kernel_spmd(nc, in_maps, core_ids=list(range(NC_)),
                                   trace=want_trace)
    except Exception:
        if not want_trace:
            raise
        res = run_bass_kernel_spmd(nc, in_maps, core_ids=list(range(NC_)),
                                   trace=False)
    LAST_EXEC_NS = res.exec_time_ns
    out = np.concatenate([res.results[k]["out"] for k in range(NC_)], axis=0)
    return out.astype(np.float32)


# revision 19
# speedup vs baseline: 1.2134x; 1.0402x over previous
"""GAT attention head (gnn_message_passing) on 8 TRN2 NeuronCores.

Strategy (dst-sharded, one AllGather):
  - Node features sharded across cores (6250 nodes each). Each core computes
    h' = x @ W for its shard plus per-node attention scalars e_dst/e_src
    (via W@a folded into an extended weight matrix), packs rows
    [h'+output_bias (128) | e_dst+b_dst | e_src+b_src | 1.0 | 0] as bf16,
    and AllGathers the full 50000-row table T.
  - Edges are sharded by destination range and sorted into 128-dst windows
    (host-side index prep). Per 128-edge chunk the core gathers the table
    rows by src ([128,1] indirect DMA - one row per partition), forms the
    score matrix F[e,j] = exp(leakyrelu(e_src_e + e_dst_j)) from the
    gathered e_src column and a per-window broadcast of the local e_dst
    values (rank-1 matmul), masks it with the one-hot dst-selection matrix
    (single fused DVE op builds (iota==dstrel)*mask), and accumulates
    Sel^T @ [msg|..|1] into a PSUM window accumulator - giving the weighted
    message sum (cols 0:128) and the softmax denominator (col 130).
  - Window epilogue: out = elu(num / max(den,1e-12)); output_bias is folded
    into the table rows (sum(alpha*(h'+bias))/den == num/den + bias).
  - No softmax max-subtraction: scores are O(1) so exp() is safe and
    softmax is shift-invariant.
Output: each core writes its 6250-row slab; host concatenates.
"""

import os
import sys

for _p in ("/opt/trn_rl_repo", "/root/.axon_site/_ro/trn_rl_repo"):
    if os.path.isdir(_p) and _p not in sys.path:
        sys.path.append(_p)

import numpy as np
import ml_dtypes

import concourse.bass as bass
import concourse.mybir as mybir
import concourse.tile as tile
from concourse import bacc
from concourse.bass import IndirectOffsetOnAxis
from concourse.bass_utils import run_bass_kernel_spmd

NC_ = 8
N = 50000
E = 800000
IN_DIM = 256
OUT_DIM = 128
NSH = N // NC_           # 6250 nodes per core
WIN = 128                # dst window size
NWIN = (NSH + WIN - 1) // WIN   # 49
TW = 132                 # table row width
F32 = mybir.dt.float32
BF16 = mybir.dt.bfloat16
I32 = mybir.dt.int32

LAST_EXEC_NS = None

_GRAPH_CACHE = {}


def _prep_edges(edge_src, edge_dst):
    """Partition edges by dst range, sort into windows, pad to a chunk
    structure (CW chunks per window) shared by all cores."""
    edge_src = np.asarray(edge_src).astype(np.int64)
    edge_dst = np.asarray(edge_dst).astype(np.int64)
    core = edge_dst // NSH
    per_core = []
    CW = np.zeros(NWIN, dtype=np.int64)
    for k in range(NC_):
        m = core == k
        s = edge_src[m]
        d = edge_dst[m] - k * NSH
        w = d // WIN
        order = np.argsort(w, kind="stable")
        per_core.append((s[order], d[order], w[order]))
        cnt = np.bincount(w, minlength=NWIN)
        CW = np.maximum(CW, (cnt + 127) // 128)
    CW = np.maximum(CW, 1)
    C = int(CW.sum())
    offs = np.zeros(NWIN + 1, dtype=np.int64)
    offs[1:] = np.cumsum(CW) * 128

    # local-src full chunks: edges whose src lies in this core's own shard
    # can gather from ag_in before the AllGather lands. Extract CL[w] =
    # min-over-cores floor(local_kw/128) full chunks per window; the chunk
    # total is unchanged (ceil identity), they just start earlier.
    CL = np.full(NWIN, 10**9, dtype=np.int64)
    for k in range(NC_):
        s, d, w = per_core[k]
        loc = (s // NSH) == k
        lcnt = np.bincount(w[loc], minlength=NWIN)
        CL = np.minimum(CL, lcnt // 128)
    CL = np.minimum(CL, CW - 1)           # keep >=1 remote chunk per window
    CL = np.maximum(CL, 0)
    CR = CW - CL
    Cl = int(CL.sum())
    Cr = int(CR.sum())
    loffs = np.zeros(NWIN + 1, dtype=np.int64)
    loffs[1:] = np.cumsum(CL) * 128
    roffs = np.zeros(NWIN + 1, dtype=np.int64)
    roffs[1:] = np.cumsum(CR) * 128

    maps = []
    for k in range(NC_):
        s, d, w = per_core[k]
        loc = (s // NSH) == k
        lsrc = np.zeros(Cl * 128, np.int32)
        ldst = np.zeros(Cl * 128, np.float32)
        lmask = np.zeros(Cl * 128, np.float32)
        srcidx = np.zeros(Cr * 128, np.int32)
        dstrel = np.zeros(Cr * 128, np.float32)
        maskv = np.zeros(Cr * 128, np.float32)
        for wv in range(NWIN):
            m = w == wv
            sw, dw = s[m], d[m]
            lw = loc[m]
            nl = int(CL[wv]) * 128
            li = np.where(lw)[0][:nl]        # exactly nl local edges
            keep = np.ones(len(sw), bool)
            keep[li] = False
            lsrc[loffs[wv]:loffs[wv] + nl] = (sw[li] - k * NSH)
            ldst[loffs[wv]:loffs[wv] + nl] = (dw[li] - wv * WIN)
            lmask[loffs[wv]:loffs[wv] + nl] = 1.0
            rs, rd = sw[keep], dw[keep]
            srcidx[roffs[wv]:roffs[wv] + len(rs)] = rs
            dstrel[roffs[wv]:roffs[wv] + len(rs)] = (rd - wv * WIN)
            maskv[roffs[wv]:roffs[wv] + len(rs)] = 1.0
        maps.append({
            "lsrcidx": np.ascontiguousarray(lsrc.reshape(Cl, 128).T),
            "ldstrel": np.ascontiguousarray(ldst.reshape(Cl, 128).T),
            "lmaskt": np.ascontiguousarray(
                (-30000.0 * (1.0 - lmask)).astype(np.float32).reshape(Cl, 128).T),
            "srcidx": np.ascontiguousarray(srcidx.reshape(Cr, 128).T),
            "dstrel": np.ascontiguousarray(dstrel.reshape(Cr, 128).T),
            "maskt": np.ascontiguousarray(
                (-30000.0 * (1.0 - maskv)).astype(np.float32).reshape(Cr, 128).T),
        })
    win_of_l = np.repeat(np.arange(NWIN), CL)
    win_of_r = np.repeat(np.arange(NWIN), CR)
    return (tuple(CL.tolist()), tuple(CR.tolist())), (Cl, Cr), \
        (win_of_l, win_of_r), maps


def _build(CLR, Cs, win_ofs):
    CL, CR = CLR
    Cl, C = Cs
    win_of_l, win_of = win_ofs
    nc = bacc.Bacc("TRN2", target_bir_lowering=False, debug=False,
                   enable_asserts=True, num_devices=NC_)
    xT = nc.dram_tensor("xT", [IN_DIM, NSH], BF16, kind="ExternalInput").ap()
    wext = nc.dram_tensor("wext", [IN_DIM, TW], BF16, kind="ExternalInput").ap()
    biast = nc.dram_tensor("biast", [128, TW], F32, kind="ExternalInput").ap()
    iota = nc.dram_tensor("iota", [128, 128], BF16, kind="ExternalInput").ap()
    ones_r = nc.dram_tensor("ones_r", [1, 128], F32, kind="ExternalInput").ap()
    lsrcidx = nc.dram_tensor("lsrcidx", [128, Cl], I32, kind="ExternalInput").ap()
    ldstrel = nc.dram_tensor("ldstrel", [128, Cl], F32, kind="ExternalInput").ap()
    lmaskt = nc.dram_tensor("lmaskt", [128, Cl], F32, kind="ExternalInput").ap()
    srcidx = nc.dram_tensor("srcidx", [128, C], I32, kind="ExternalInput").ap()
    dstrel = nc.dram_tensor("dstrel", [128, C], F32, kind="ExternalInput").ap()
    maskt = nc.dram_tensor("maskt", [128, C], F32, kind="ExternalInput").ap()
    out = nc.dram_tensor("out", [NSH, OUT_DIM], F32, kind="ExternalOutput").ap()

    ag_in = nc.dram_tensor("ag_in", [NSH, TW], BF16)
    edloc = nc.dram_tensor("edloc", [NWIN * WIN, 1], F32)   # padded e_dst column
    T = nc.dram_tensor("t_full", [N, TW], BF16, addr_space="Shared")

    EXP = mybir.ActivationFunctionType.Exp
    AO = mybir.AluOpType
    NT = NWIN  # node tiles of 128 in this core's shard (48*128 + 106)

    first_of = {}
    last_of = {}
    for c, w in enumerate(win_of):
        if w not in first_of:
            first_of[w] = c
        last_of[w] = c
    lfirst_of = {}
    llast_of = {}
    for c, w in enumerate(win_of_l):
        if w not in lfirst_of:
            lfirst_of[w] = c
        llast_of[w] = c

    with tile.TileContext(nc) as tc:
        with tc.tile_pool(name="const", bufs=1) as constp, \
             tc.tile_pool(name="idx", bufs=1) as idxp:
            wext_t = constp.tile([128, 2 * TW], BF16)
            nc.sync.dma_start(wext_t[:, 0:TW], wext[0:128, :])
            nc.sync.dma_start(wext_t[:, TW:2 * TW], wext[128:256, :])
            biast_t = constp.tile([128, TW], F32)
            nc.sync.dma_start(biast_t[:], biast[:, :])
            iota_t = constp.tile([128, 128], BF16)
            nc.sync.dma_start(iota_t[:], iota[:, :])
            ones_t = constp.tile([1, 128], F32)
            nc.sync.dma_start(ones_t[:], ones_r[:, :])
            lsrcidx_t = idxp.tile([128, Cl], I32)
            nc.sync.dma_start(lsrcidx_t[:], lsrcidx[:, :])
            ldstrel_t = idxp.tile([128, Cl], F32)
            nc.sync.dma_start(ldstrel_t[:], ldstrel[:, :])
            lmask_t = idxp.tile([128, Cl], F32)
            nc.sync.dma_start(lmask_t[:], lmaskt[:, :])
            srcidx_t = idxp.tile([128, C], I32)
            nc.sync.dma_start(srcidx_t[:], srcidx[:, :])
            dstrel_t = idxp.tile([128, C], F32)
            nc.sync.dma_start(dstrel_t[:], dstrel[:, :])
            mask_t = idxp.tile([128, C], F32)
            nc.sync.dma_start(mask_t[:], maskt[:, :])

            # ---- phase 1: h' + table build + AllGather ----
            with tc.tile_pool(name="p1x", bufs=1) as p1x, \
                 tc.tile_pool(name="p1t", bufs=3) as p1t, \
                 tc.tile_pool(name="ps1", bufs=4, space="PSUM") as ps1:
                xt = p1x.tile([128, 2 * NSH], BF16)
                nc.sync.dma_start(xt[:, 0:NSH], xT[0:128, :])
                nc.sync.dma_start(xt[:, NSH:2 * NSH], xT[128:256, :])
                edcols = p1x.tile([128, NWIN], F32)
                nc.vector.memset(edcols[:], 0.0)
                # four independent table-block tiles so each block's ag_in
                # write can start as soon as ITS adds are done (a single big
                # tile serializes the write behind all 49 adds)
                blk_base = [0, 13, 25, 37]
                blk_len = [13, 12, 12, 12]
                tb4 = [p1x.tile([128, blk_len[b] * TW], BF16, name=f"tb4_{b}",
                                tag=f"tb4_{b}") for b in range(4)]
                for m in range(NT):
                    pm = min(128, NSH - m * 128)
                    b = 0
                    while m >= blk_base[b] + blk_len[b]:
                        b += 1
                    lm = m - blk_base[b]
                    ps = ps1.tile([128, TW], F32, tag="ps")
                    nc.tensor.matmul(out=ps[:pm, :],
                                     lhsT=xt[:, m * 128: m * 128 + pm],
                                     rhs=wext_t[:, 0:TW], start=True, stop=False)
                    nc.tensor.matmul(out=ps[:pm, :],
                                     lhsT=xt[:, NSH + m * 128: NSH + m * 128 + pm],
                                     rhs=wext_t[:, TW:2 * TW], start=False, stop=True)
                    nc.vector.tensor_tensor(tb4[b][:pm, lm * TW:(lm + 1) * TW],
                                            ps[:pm, :], biast_t[:pm, :], op=AO.add)
                    nc.vector.tensor_tensor(edcols[:pm, m:m + 1], ps[:pm, 128:129],
                                            biast_t[:pm, 128:129], op=AO.add)
                    if m == blk_base[b] + blk_len[b] - 1:
                        # block complete: write its full-128-row tiles; the
                        # 106-row tile 48 in the last block goes separately
                        nfull = blk_len[b] - (1 if b == 3 else 0)
                        nc.sync.dma_start(
                            ag_in.ap()[blk_base[b] * 128:
                                       (blk_base[b] + nfull) * 128, :].rearrange(
                                "(m p) e -> p m e", p=128),
                            tb4[b][:].rearrange(
                                "p (m e) -> p m e", e=TW)[:, 0:nfull, :])
                nc.sync.dma_start(ag_in[(NWIN - 1) * 128:NSH, :],
                                  tb4[3][:106, 11 * TW:12 * TW])
                # node m*128+p lives at edcols[p, m]; edloc is node-flat
                nc.sync.dma_start(
                    edloc.ap().rearrange("(m p) one -> p (m one)", p=128),
                    edcols[:])

            nc.gpsimd.collective_compute(
                "AllGather", AO.bypass,
                replica_groups=[list(range(NC_))],
                ins=[ag_in.ap().opt()],
                outs=[T.ap().opt()],
            )

            # ---- phases 2+3: gather, score, accumulate, evacuate ----
            with tc.tile_pool(name="gath", bufs=16) as gp, \
                 tc.tile_pool(name="wrow", bufs=3) as wrp, \
                 tc.tile_pool(name="wbc", bufs=3) as wbp, \
                 tc.tile_pool(name="sc", bufs=8) as scp, \
                 tc.tile_pool(name="accp", bufs=1) as accp, \
                 tc.tile_pool(name="psB", bufs=2, space="PSUM") as psB, \
                 tc.tile_pool(name="ps2", bufs=3, space="PSUM") as ps2, \
                 tc.tile_pool(name="evac", bufs=2) as ev:
                # phase L: chunks whose srcs are all in this core's own shard
                # gather from ag_in and run while the AllGather is in flight;
                # their window partials land in SBUF accumulators.
                accs = {}
                psw = None
                edw_b = None
                for c in range(Cl):
                    w = int(win_of_l[c])
                    if lfirst_of[w] == c:
                        edr = wrp.tile([1, WIN], F32, tag="edr")
                        edloc_rows = edloc.ap().rearrange(
                            "(a b) one -> a (b one)", b=WIN)
                        nc.sync.dma_start(edr[:], edloc_rows[w:w + 1, :])
                        edp = psB.tile([128, WIN], F32, tag="edp")
                        nc.tensor.matmul(out=edp[:], lhsT=ones_t[:], rhs=edr[:],
                                         start=True, stop=True)
                        edw_b = wbp.tile([128, WIN], F32, tag="edw")
                        nc.vector.tensor_copy(edw_b[:], edp[:])
                        psw = ps2.tile([128, TW], F32, tag="psw")
                    msg = gp.tile([128, TW], BF16, tag="msg")
                    nc.gpsimd.indirect_dma_start(
                        out=msg[:], out_offset=None, in_=ag_in.ap(),
                        in_offset=IndirectOffsetOnAxis(
                            ap=lsrcidx_t[:, c: c + 1], axis=0))
                    esf = scp.tile([128, 1], F32, tag="esf")
                    nc.vector.tensor_copy(esf[:], msg[:, 129:130])
                    s0 = scp.tile([128, WIN], F32, tag="s0")
                    nc.vector.tensor_scalar(s0[:], edw_b[:], esf[:, 0:1],
                                            lmask_t[:, c: c + 1],
                                            op0=AO.add, op1=AO.add)
                    s1 = scp.tile([128, WIN], F32, tag="s1")
                    nc.vector.scalar_tensor_tensor(s1[:], s0[:], 0.2, s0[:],
                                                   op0=AO.mult, op1=AO.max)
                    fm = scp.tile([128, WIN], BF16, tag="fm")
                    nc.scalar.activation(fm[:], s1[:], EXP)
                    selw = scp.tile([128, WIN], BF16, tag="selw")
                    nc.vector.scalar_tensor_tensor(selw[:], iota_t[:],
                                                   ldstrel_t[:, c: c + 1],
                                                   fm[:], op0=AO.is_equal,
                                                   op1=AO.mult)
                    nc.tensor.matmul(out=psw[:], lhsT=selw[:], rhs=msg[:],
                                     start=(lfirst_of[w] == c),
                                     stop=(llast_of[w] == c))
                    if llast_of[w] == c:
                        acc = accp.tile([128, TW], F32, name=f"acc_{w}",
                                        tag=f"acc_{w}")
                        nc.vector.tensor_copy(acc[:], psw[:])
                        accs[w] = acc
                psw = None
                edw_b = None
                for c in range(C):
                    w = int(win_of[c])
                    if first_of[w] == c:
                        # per-window: broadcast e_dst row to all partitions
                        edr = wrp.tile([1, WIN], F32, tag="edr")
                        edloc_rows = edloc.ap().rearrange(
                            "(a b) one -> a (b one)", b=WIN)
                        nc.sync.dma_start(edr[:], edloc_rows[w:w + 1, :])
                        edp = psB.tile([128, WIN], F32, tag="edp")
                        nc.tensor.matmul(out=edp[:], lhsT=ones_t[:], rhs=edr[:],
                                         start=True, stop=True)
                        edw_b = wbp.tile([128, WIN], F32, tag="edw")
                        nc.vector.tensor_copy(edw_b[:], edp[:])
                        psw = ps2.tile([128, TW], F32, tag="psw")
                    # per-chunk: gather 128 table rows by src
                    msg = gp.tile([128, TW], BF16, tag="msg")
                    nc.gpsimd.indirect_dma_start(
                        out=msg[:], out_offset=None, in_=T.ap(),
                        in_offset=IndirectOffsetOnAxis(
                            ap=srcidx_t[:, c: c + 1], axis=0))
                    # scores: F = exp(leaky(e_src_e + e_dst_j + maskbias_e))
                    esf = scp.tile([128, 1], F32, tag="esf")
                    nc.vector.tensor_copy(esf[:], msg[:, 129:130])
                    s0 = scp.tile([128, WIN], F32, tag="s0")
                    nc.vector.tensor_scalar(s0[:], edw_b[:], esf[:, 0:1],
                                            mask_t[:, c: c + 1],
                                            op0=AO.add, op1=AO.add)
                    s1 = scp.tile([128, WIN], F32, tag="s1")
                    nc.vector.scalar_tensor_tensor(s1[:], s0[:], 0.2, s0[:],
                                                   op0=AO.mult, op1=AO.max)
                    fm = scp.tile([128, WIN], BF16, tag="fm")
                    nc.scalar.activation(fm[:], s1[:], EXP)
                    selw = scp.tile([128, WIN], BF16, tag="selw")
                    nc.vector.scalar_tensor_tensor(selw[:], iota_t[:],
                                                   dstrel_t[:, c: c + 1],
                                                   fm[:], op0=AO.is_equal,
                                                   op1=AO.mult)
                    nc.tensor.matmul(out=psw[:], lhsT=selw[:], rhs=msg[:],
                                     start=(first_of[w] == c),
                                     stop=(last_of[w] == c))
                    if last_of[w] == c:
                        pw = min(128, NSH - w * 128)
                        if w in accs:
                            tot = ev.tile([128, TW], F32, tag="tot")
                            nc.vector.tensor_tensor(tot[:], psw[:],
                                                    accs[w][:], op=AO.add)
                            srcv = tot
                        else:
                            srcv = psw
                        den = ev.tile([128, 1], F32, tag="den")
                        nc.vector.tensor_scalar(den[:], srcv[:, 130:131],
                                                1e-12, None, op0=AO.max)
                        rec = ev.tile([128, 1], F32, tag="rec")
                        nc.vector.reciprocal(rec[:], den[:])
                        o1 = ev.tile([128, 128], F32, tag="o1")
                        nc.vector.tensor_scalar(o1[:], srcv[:, 0:128],
                                                rec[:, 0:1], None, op0=AO.mult)
                        mng = ev.tile([128, 128], F32, tag="mng")
                        nc.vector.tensor_scalar(mng[:], o1[:], 0.0, None,
                                                op0=AO.min)
                        eng = ev.tile([128, 128], F32, tag="eng")
                        nc.scalar.activation(eng[:], mng[:], EXP)
                        fin = ev.tile([128, 128], F32, tag="fin")
                        nc.vector.scalar_tensor_tensor(fin[:], o1[:], 0.0,
                                                       eng[:], op0=AO.max,
                                                       op1=AO.add)
                        fin2 = ev.tile([128, 128], F32, tag="fin2")
                        nc.vector.tensor_scalar(fin2[:], fin[:], 1.0, None,
                                                op0=AO.subtract)
                        nc.sync.dma_start(out[w * 128: w * 128 + pw, :],
                                          fin2[:pw, :])
    nc.compile()
    return nc


def _host_inputs(inputs):
    x = np.ascontiguousarray(np.asarray(inputs["inputs"], dtype=np.float32))
    edge_src = np.asarray(inputs["edge_src"])
    edge_dst = np.asarray(inputs["edge_dst"])
    W = np.asarray(inputs["W_seq"], dtype=np.float32)
    a_dst = np.asarray(inputs["a_dst"], dtype=np.float32)
    b_dst = np.float32(inputs["b_dst"])
    a_src = np.asarray(inputs["a_src"], dtype=np.float32)
    b_src = np.float32(inputs["b_src"])
    output_bias = np.asarray(inputs["output_bias"], dtype=np.float32)

    CLR, Cs, win_ofs, edge_maps = _prep_edges(edge_src, edge_dst)

    wext = np.zeros((IN_DIM, TW), np.float32)
    wext[:, 0:OUT_DIM] = W
    wext[:, 128] = W @ a_dst
    wext[:, 129] = W @ a_src
    wext = wext.astype(ml_dtypes.bfloat16)
    bias_ext = np.zeros(TW, np.float32)
    bias_ext[0:OUT_DIM] = output_bias
    bias_ext[128] = b_dst
    bias_ext[129] = b_src
    bias_ext[130] = 1.0
    biast = np.ascontiguousarray(np.tile(bias_ext[None, :], (128, 1)))
    iota = np.ascontiguousarray(
        np.tile(np.arange(128, dtype=np.float32)[None, :], (128, 1))
    ).astype(ml_dtypes.bfloat16)
    ones_r = np.ones((1, 128), np.float32)

    in_maps = []
    for k in range(NC_):
        m = {
            "xT": np.ascontiguousarray(
                x[k * NSH:(k + 1) * NSH].T).astype(ml_dtypes.bfloat16),
            "wext": wext,
            "biast": biast,
            "iota": iota,
            "ones_r": ones_r,
        }
        m.update(edge_maps[k])
        in_maps.append(m)
    return CLR, Cs, win_ofs, in_maps


def kernel(**inputs) -> np.ndarray:
    global LAST_EXEC_NS
    CLR, Cs, win_ofs, in_maps = _host_inputs(inputs)
    key = (CLR, Cs)
    if key not in _GRAPH_CACHE:
        _GRAPH_CACHE[key] = _build(CLR, Cs, win_ofs)
    nc = _GRAPH_CACHE[key]

    want_trace = bool(int(os.environ.get("KERNEL_TRACE", "0")))
    try:
        res = run_bass_kernel_spmd(nc, in_maps, core_ids=list(range(NC_)),
                                   trace=want_trace)
    except Exception:
        if not want_trace:
            raise
        res = run_bass_kernel_spmd(nc, in_maps, core_ids=list(range(NC_)),
                                   trace=False)
    LAST_EXEC_NS = res.exec_time_ns
    out = np.concatenate([res.results[k]["out"] for k in range(NC_)], axis=0)
    return out.astype(np.float32)
